# revision 28
# baseline (speedup 1.0000x reference)
"""Trainium2 Bass kernel: scatter rho[b, i, j] -> out[b, fock_idx[i], fock_idx[j]].

Sharding: batch dim B across the 8 NeuronCores (pure data parallel). fock_idx is
known on the host at call time, so the scatter addressing is baked into the
compiled program as static DMA/compute access patterns.

Per-core algorithm (out is [D, D], zero except out[idx[i], idx[j]] = rho[i, j]):
  - The runtime hands the NEFF a zero-initialized ExternalOutput buffer, so
    only rows/columns that receive data are written: each rho row expands to a
    [span]-wide SBUF row (runs at target offsets, zeros in gaps); row-runs of
    32 are stored with one DMA each, touching only columns [c0, c1).
  - All DMA rides the two HWDGE rings (SP and ACT); SWDGE is avoided so
    GpSimd is free for copies. Tiles are processed in groups of 2; each
    group's two half-loads go to DIFFERENT rings so the group's data lands in
    one ring-drain period and its expansion can start early.
  - Expansion copies: ALL on Vector, one 4-dim-AP instruction per run-pair
    covering both tiles of a group ([128 parts][2 tiles][2 runs][32]).
    Measured: solo Vector copies retire in ~190-230 ns, while splitting work
    with GpSimd stalls both engines multi-us at a time (shared SBUF ports;
    also 4-dim APs on GpSimd pull in a ~3 us larger Q7 ucode download that
    delays the start barrier).
  - THREE W buffers (group 3 reuses W0; gap columns stay zero across reuse
    since copies touch only data columns). W0/W1 memsets (Vector + GpSimd)
    finish before Vector's first copy burst — GpSimd never memsets
    concurrently with Vector copies (shared-SBUF-port stalls cost ~2 us per
    collision). W2 is zeroed by Vector itself in its idle window after
    group 1's copies, so group 2 never waits on group 0's store drain.
  - Stores alternate rings behind the loads in each ring's FIFO; the wire
    stays saturated from first load byte to last store byte.
"""

import numpy as np

import concourse.bacc as bacc
import concourse.bass as bass
import concourse.mybir as mybir
from concourse import tile
from concourse.bass_utils import run_bass_kernel_spmd

N_CORES = 8
P = 128  # SBUF partitions
GT = 2   # tiles per group


def _runs(dst, src):
    """Maximal runs where dst and src both advance by 1. Yields (d0, s0, len)."""
    out = []
    d0, s0, L = int(dst[0]), int(src[0]), 1
    for k in range(1, len(dst)):
        if int(dst[k]) == d0 + L and int(src[k]) == s0 + L:
            L += 1
        else:
            out.append((d0, s0, L))
            d0, s0, L = int(dst[k]), int(src[k]), 1
    out.append((d0, s0, L))
    return out


def _pair_runs(col_runs):
    """Group adjacent equal-length runs into stride-2 pairs.

    Returns (dst0, src0, pair_dst_stride, pair_src_stride, n, L) with n in
    {1, 2} repeats of an L-wide copy.
    """
    out = []
    k = 0
    while k < len(col_runs):
        d0, s0, L = col_runs[k]
        if k + 1 < len(col_runs) and col_runs[k + 1][2] == L:
            d1, s1, _ = col_runs[k + 1]
            out.append((d0, s0, d1 - d0, s1 - s0, 2, L))
            k += 2
        else:
            out.append((d0, s0, L, L, 1, L))
            k += 1
    return out


def _build(idx, D, n):
    """Build the per-core Bass program with idx baked in."""
    f32 = mybir.dt.float32

    order = np.argsort(idx, kind="stable")
    col_runs = _runs(idx[order], order)  # (dst_col, src_col, len)
    c0 = min(r[0] for r in col_runs)
    c1 = max(r[0] + r[2] for r in col_runs)
    # Widen the span so every store row is 64-byte aligned at both ends
    # (out row starts at (r*D + c0)*4; D is a multiple of 16 here, so
    # aligning c0/c1 to 16 elements aligns every row). Unaligned 8 KB HBM
    # writes pay a read-modify-write on the edge atoms of every row.
    if D % 16 == 0:
        c0 = (c0 // 16) * 16
        c1 = ((c1 + 15) // 16) * 16
    span = c1 - c0
    pairs = _pair_runs(col_runs)

    nc = bacc.Bacc("TRN2", target_bir_lowering=False, debug=False,
                   num_devices=N_CORES, enable_partition_id=False)
    rho = nc.dram_tensor("rho", [n, n], f32, kind="ExternalInput")
    out = nc.dram_tensor("out", [D, D], f32, kind="ExternalOutput")

    n_tiles = (n + P - 1) // P
    n_groups = (n_tiles + GT - 1) // GT
    with tile.TileContext(nc) as tc:
        with (
            tc.tile_pool(name="rp", bufs=1) as rp,
            tc.tile_pool(name="wp", bufs=1) as wp,
        ):
            n_wbufs = min(3, n_groups)
            Rs = [rp.tile([P, GT * n], f32, name=f"R{g}")
                  for g in range(n_groups)]
            Ws = [wp.tile([P, GT * span], f32, name=f"W{g}")
                  for g in range(n_wbufs)]

            # All loads up front; a group's two halves ride DIFFERENT rings
            # so each group completes one ring-drain period after the last.
            for g in range(n_groups):
                for j in range(GT):
                    ring = nc.sync if j == 0 else nc.scalar
                    r0 = (g * GT + j) * P
                    rows = min(P, n - r0)
                    ring.dma_start(Rs[g][:rows, j * n:j * n + n],
                                   rho[r0:r0 + rows, :])

            # Gap zeroing: W0/W1 memsets all complete before Vector's
            # first copy burst (GpSimd never memsets concurrently with
            # Vector copies — shared-SBUF-port stalls). W2 is zeroed by
            # Vector itself in its natural idle window after group 1's
            # copies, so group 2 does not wait on group 0's store drain.
            nc.vector.memset(Ws[0][:, 0:span], 0.0)
            nc.gpsimd.memset(Ws[0][:, span:GT * span], 0.0)
            if n_wbufs > 1:
                nc.gpsimd.memset(Ws[1][:, 0:span], 0.0)
                nc.gpsimd.memset(Ws[1][:, span:GT * span], 0.0)

            n_store = 0
            for g in range(n_groups):
                W, R = Ws[g % n_wbufs], Rs[g]

                if g == 2 and n_wbufs > 2:
                    nc.vector.memset(Ws[2][:, 0:span], 0.0)
                    nc.vector.memset(Ws[2][:, span:GT * span], 0.0)

                # One 4-dim Vector copy per run-pair covers both tiles of
                # the group ([128 parts][GT tiles][2 runs][width]).
                for d0, s0, ds, ss, cnt, L in pairs:
                    dst = bass.AP(W.tensor, W.offset + (d0 - c0),
                                  [[W.ap[0][0], P], [span, GT],
                                   [ds, cnt], [1, L]])
                    src = bass.AP(R.tensor, R.offset + s0,
                                  [[R.ap[0][0], P], [n, GT],
                                   [ss, cnt], [1, L]])
                    nc.vector.tensor_copy(dst, src)

                # Row runs: consecutive rho rows with consecutive target rows
                # share one store DMA, alternating between the two rings.
                # (Routing stores through SWDGE as a third queue measured
                # ~10 us WORSE: Q7 descriptor generation serializes; merging
                # row-run pairs into 3-dim-AP stores lowered incorrectly.)
                for j in range(GT):
                    r0 = (g * GT + j) * P
                    rows = min(P, n - r0)
                    for dr, sr, L in _runs(idx[r0:r0 + rows], range(rows)):
                        ring = nc.sync if n_store % 2 == 0 else nc.scalar
                        n_store += 1
                        ring.dma_start(out[dr:dr + L, c0:c1],
                                       W[sr:sr + L, j * span:j * span + span])
    nc.compile()
    return nc


def kernel(input_state, fock_idx, fock_dim):
    input_state = np.asarray(input_state)
    idx = np.asarray(fock_idx).astype(np.int64)
    D = int(fock_dim)
    B, n, _ = input_state.shape

    nc = _build(idx, D, n)

    out = np.empty((B, D, D), dtype=input_state.dtype)
    for start in range(0, B, N_CORES):
        stop = min(start + N_CORES, B)
        in_maps = [
            {"rho": np.ascontiguousarray(input_state[b], dtype=np.float32)}
            for b in range(start, stop)
        ]
        res = run_bass_kernel_spmd(nc, in_maps,
                                   core_ids=list(range(stop - start)))
        for k, b in enumerate(range(start, stop)):
            out[b] = res.results[k]["out"]
    return out


# revision 29
# speedup vs baseline: 1.0744x; 1.0744x over previous
"""Trainium2 Bass kernel: scatter rho[b, i, j] -> out[b, fock_idx[i], fock_idx[j]].

Sharding: batch dim B across the 8 NeuronCores (pure data parallel). fock_idx is
known on the host at call time, so the scatter addressing is baked into the
compiled program as static DMA/compute access patterns.

Per-core algorithm (out is [D, D], zero except out[idx[i], idx[j]] = rho[i, j]):
  - The runtime hands the NEFF a zero-initialized ExternalOutput buffer, so
    only rows/columns that receive data are written: each rho row expands to a
    [span]-wide SBUF row (runs at target offsets, zeros in gaps); row-runs of
    32 are stored with one DMA each, touching only columns [c0, c1).
  - All DMA rides the two HWDGE rings (SP and ACT); SWDGE is avoided so
    GpSimd is free for copies. Tiles are processed in groups of 2; each
    group's two half-loads go to DIFFERENT rings so the group's data lands in
    one ring-drain period and its expansion can start early.
  - Expansion copies: ALL on Vector, one 4-dim-AP instruction per run-pair
    covering both tiles of a group ([128 parts][2 tiles][2 runs][32]).
    Measured: solo Vector copies retire in ~190-230 ns, while splitting work
    with GpSimd stalls both engines multi-us at a time (shared SBUF ports;
    also 4-dim APs on GpSimd pull in a ~3 us larger Q7 ucode download that
    delays the start barrier).
  - THREE W buffers (group 3 reuses W0; gap columns stay zero across reuse
    since copies touch only data columns). W0/W1 memsets (Vector + GpSimd)
    finish before Vector's first copy burst — GpSimd never memsets
    concurrently with Vector copies (shared-SBUF-port stalls cost ~2 us per
    collision). W2 is zeroed by Vector itself in its idle window after
    group 1's copies, so group 2 never waits on group 0's store drain.
  - Stores alternate rings behind the loads in each ring's FIFO; the wire
    stays saturated from first load byte to last store byte.
"""

import numpy as np

import concourse.bacc as bacc
import concourse.bass as bass
import concourse.mybir as mybir
from concourse import tile
from concourse.bass_utils import run_bass_kernel_spmd

N_CORES = 8
P = 128  # SBUF partitions
GT = 2   # tiles per group


def _runs(dst, src):
    """Maximal runs where dst and src both advance by 1. Yields (d0, s0, len)."""
    out = []
    d0, s0, L = int(dst[0]), int(src[0]), 1
    for k in range(1, len(dst)):
        if int(dst[k]) == d0 + L and int(src[k]) == s0 + L:
            L += 1
        else:
            out.append((d0, s0, L))
            d0, s0, L = int(dst[k]), int(src[k]), 1
    out.append((d0, s0, L))
    return out


def _pair_runs(col_runs):
    """Group adjacent equal-length runs into stride-2 pairs.

    Returns (dst0, src0, pair_dst_stride, pair_src_stride, n, L) with n in
    {1, 2} repeats of an L-wide copy.
    """
    out = []
    k = 0
    while k < len(col_runs):
        d0, s0, L = col_runs[k]
        if k + 1 < len(col_runs) and col_runs[k + 1][2] == L:
            d1, s1, _ = col_runs[k + 1]
            out.append((d0, s0, d1 - d0, s1 - s0, 2, L))
            k += 2
        else:
            out.append((d0, s0, L, L, 1, L))
            k += 1
    return out


def _build(idx, D, n):
    """Build the per-core Bass program with idx baked in."""
    f32 = mybir.dt.float32

    order = np.argsort(idx, kind="stable")
    col_runs = _runs(idx[order], order)  # (dst_col, src_col, len)
    c0 = min(r[0] for r in col_runs)
    c1 = max(r[0] + r[2] for r in col_runs)
    # Widen the span so every store row is 64-byte aligned at both ends
    # (out row starts at (r*D + c0)*4; D is a multiple of 16 here, so
    # aligning c0/c1 to 16 elements aligns every row). Unaligned 8 KB HBM
    # writes pay a read-modify-write on the edge atoms of every row.
    if D % 16 == 0:
        c0 = (c0 // 16) * 16
        c1 = ((c1 + 15) // 16) * 16
    span = c1 - c0
    pairs = _pair_runs(col_runs)

    nc = bacc.Bacc("TRN2", target_bir_lowering=False, debug=False,
                   num_devices=N_CORES, enable_partition_id=False)
    rho = nc.dram_tensor("rho", [n, n], f32, kind="ExternalInput")
    out = nc.dram_tensor("out", [D, D], f32, kind="ExternalOutput")

    n_tiles = (n + P - 1) // P
    n_groups = (n_tiles + GT - 1) // GT
    with tile.TileContext(nc) as tc:
        with (
            tc.tile_pool(name="rp", bufs=1) as rp,
            tc.tile_pool(name="wp", bufs=1) as wp,
        ):
            n_wbufs = min(3, n_groups)
            Rs = [rp.tile([P, GT * n], f32, name=f"R{g}")
                  for g in range(n_groups)]
            Ws = [wp.tile([P, GT * span], f32, name=f"W{g}")
                  for g in range(n_wbufs)]

            # Loads mostly up front; a group's two halves ride DIFFERENT
            # rings so each group completes one ring-drain period after the
            # last. The LAST group's loads are emitted after group 0's
            # stores, so the first stores interleave with the load tail
            # (mixed reads+writes beat the write-only ring rate).
            def issue_load(g):
                for j in range(GT):
                    ring = nc.sync if j == 0 else nc.scalar
                    r0 = (g * GT + j) * P
                    rows = min(P, n - r0)
                    ring.dma_start(Rs[g][:rows, j * n:j * n + n],
                                   rho[r0:r0 + rows, :])

            for g in range(max(1, n_groups - 1)):
                issue_load(g)

            # Gap zeroing: W0/W1 memsets all complete before Vector's
            # first copy burst (GpSimd never memsets concurrently with
            # Vector copies — shared-SBUF-port stalls). W2 is zeroed by
            # Vector itself in its natural idle window after group 1's
            # copies, so group 2 does not wait on group 0's store drain.
            nc.vector.memset(Ws[0][:, 0:span], 0.0)
            nc.gpsimd.memset(Ws[0][:, span:GT * span], 0.0)
            if n_wbufs > 1:
                nc.gpsimd.memset(Ws[1][:, 0:span], 0.0)
                nc.gpsimd.memset(Ws[1][:, span:GT * span], 0.0)

            n_store = 0
            for g in range(n_groups):
                W, R = Ws[g % n_wbufs], Rs[g]

                if g == 2 and n_wbufs > 2:
                    nc.vector.memset(Ws[2][:, 0:span], 0.0)
                    nc.vector.memset(Ws[2][:, span:GT * span], 0.0)

                # One 4-dim Vector copy per run-pair covers both tiles of
                # the group ([128 parts][GT tiles][2 runs][width]).
                for d0, s0, ds, ss, cnt, L in pairs:
                    dst = bass.AP(W.tensor, W.offset + (d0 - c0),
                                  [[W.ap[0][0], P], [span, GT],
                                   [ds, cnt], [1, L]])
                    src = bass.AP(R.tensor, R.offset + s0,
                                  [[R.ap[0][0], P], [n, GT],
                                   [ss, cnt], [1, L]])
                    nc.vector.tensor_copy(dst, src)

                # Row runs: consecutive rho rows with consecutive target rows
                # share one store DMA, alternating between the two rings.
                # (Routing stores through SWDGE as a third queue measured
                # ~10 us WORSE: Q7 descriptor generation serializes; merging
                # row-run pairs into 3-dim-AP stores lowered incorrectly.)
                for j in range(GT):
                    r0 = (g * GT + j) * P
                    rows = min(P, n - r0)
                    for dr, sr, L in _runs(idx[r0:r0 + rows], range(rows)):
                        ring = nc.sync if n_store % 2 == 0 else nc.scalar
                        n_store += 1
                        ring.dma_start(out[dr:dr + L, c0:c1],
                                       W[sr:sr + L, j * span:j * span + span])

                if g == 0 and n_groups > 1:
                    issue_load(n_groups - 1)
    nc.compile()
    return nc


def kernel(input_state, fock_idx, fock_dim):
    input_state = np.asarray(input_state)
    idx = np.asarray(fock_idx).astype(np.int64)
    D = int(fock_dim)
    B, n, _ = input_state.shape

    nc = _build(idx, D, n)

    out = np.empty((B, D, D), dtype=input_state.dtype)
    for start in range(0, B, N_CORES):
        stop = min(start + N_CORES, B)
        in_maps = [
            {"rho": np.ascontiguousarray(input_state[b], dtype=np.float32)}
            for b in range(start, stop)
        ]
        res = run_bass_kernel_spmd(nc, in_maps,
                                   core_ids=list(range(stop - start)))
        for k, b in enumerate(range(start, stop)):
            out[b] = res.results[k]["out"]
    return out


# revision 30
# speedup vs baseline: 1.1179x; 1.0405x over previous
"""Trainium2 Bass kernel: scatter rho[b, i, j] -> out[b, fock_idx[i], fock_idx[j]].

Sharding: batch dim B across the 8 NeuronCores (pure data parallel). fock_idx is
known on the host at call time, so the scatter addressing is baked into the
compiled program as static DMA/compute access patterns.

Per-core algorithm (out is [D, D], zero except out[idx[i], idx[j]] = rho[i, j]):
  - The runtime hands the NEFF a zero-initialized ExternalOutput buffer, so
    only rows/columns that receive data are written: each rho row expands to a
    [span]-wide SBUF row (runs at target offsets, zeros in gaps); row-runs of
    32 are stored with one DMA each, touching only columns [c0, c1).
  - All DMA rides the two HWDGE rings (SP and ACT); SWDGE is avoided so
    GpSimd is free for copies. Tiles are processed in groups of 2; each
    group's two half-loads go to DIFFERENT rings so the group's data lands in
    one ring-drain period and its expansion can start early.
  - Expansion copies: ALL on Vector, one 4-dim-AP instruction per run-pair
    covering both tiles of a group ([128 parts][2 tiles][2 runs][32]).
    Measured: solo Vector copies retire in ~190-230 ns, while splitting work
    with GpSimd stalls both engines multi-us at a time (shared SBUF ports;
    also 4-dim APs on GpSimd pull in a ~3 us larger Q7 ucode download that
    delays the start barrier).
  - THREE W buffers (group 3 reuses W0; gap columns stay zero across reuse
    since copies touch only data columns). W0/W1 memsets (Vector + GpSimd)
    finish before Vector's first copy burst — GpSimd never memsets
    concurrently with Vector copies (shared-SBUF-port stalls cost ~2 us per
    collision). W2 is zeroed by Vector itself in its idle window after
    group 1's copies, so group 2 never waits on group 0's store drain.
  - Stores alternate rings behind the loads in each ring's FIFO; the wire
    stays saturated from first load byte to last store byte.
"""

import numpy as np

import concourse.bacc as bacc
import concourse.bass as bass
import concourse.mybir as mybir
from concourse import tile
from concourse.bass_utils import run_bass_kernel_spmd

N_CORES = 8
P = 128  # SBUF partitions
GT = 2   # tiles per group


def _runs(dst, src):
    """Maximal runs where dst and src both advance by 1. Yields (d0, s0, len)."""
    out = []
    d0, s0, L = int(dst[0]), int(src[0]), 1
    for k in range(1, len(dst)):
        if int(dst[k]) == d0 + L and int(src[k]) == s0 + L:
            L += 1
        else:
            out.append((d0, s0, L))
            d0, s0, L = int(dst[k]), int(src[k]), 1
    out.append((d0, s0, L))
    return out


def _pair_runs(col_runs):
    """Group adjacent equal-length runs into stride-2 pairs.

    Returns (dst0, src0, pair_dst_stride, pair_src_stride, n, L) with n in
    {1, 2} repeats of an L-wide copy.
    """
    out = []
    k = 0
    while k < len(col_runs):
        d0, s0, L = col_runs[k]
        if k + 1 < len(col_runs) and col_runs[k + 1][2] == L:
            d1, s1, _ = col_runs[k + 1]
            out.append((d0, s0, d1 - d0, s1 - s0, 2, L))
            k += 2
        else:
            out.append((d0, s0, L, L, 1, L))
            k += 1
    return out


def _build(idx, D, n):
    """Build the per-core Bass program with idx baked in."""
    f32 = mybir.dt.float32

    order = np.argsort(idx, kind="stable")
    col_runs = _runs(idx[order], order)  # (dst_col, src_col, len)
    c0 = min(r[0] for r in col_runs)
    c1 = max(r[0] + r[2] for r in col_runs)
    # Widen the span so every store row is 64-byte aligned at both ends
    # (out row starts at (r*D + c0)*4; D is a multiple of 16 here, so
    # aligning c0/c1 to 16 elements aligns every row). Unaligned 8 KB HBM
    # writes pay a read-modify-write on the edge atoms of every row.
    if D % 16 == 0:
        c0 = (c0 // 16) * 16
        c1 = ((c1 + 15) // 16) * 16
    span = c1 - c0
    pairs = _pair_runs(col_runs)

    nc = bacc.Bacc("TRN2", target_bir_lowering=False, debug=False,
                   num_devices=N_CORES, enable_partition_id=False)
    rho = nc.dram_tensor("rho", [n, n], f32, kind="ExternalInput")
    out = nc.dram_tensor("out", [D, D], f32, kind="ExternalOutput")

    n_tiles = (n + P - 1) // P
    n_groups = (n_tiles + GT - 1) // GT
    with tile.TileContext(nc) as tc:
        with (
            tc.tile_pool(name="rp", bufs=1) as rp,
            tc.tile_pool(name="wp", bufs=1) as wp,
        ):
            n_wbufs = min(3, n_groups)
            Rs = [rp.tile([P, GT * n], f32, name=f"R{g}")
                  for g in range(n_groups)]
            Ws = [wp.tile([P, GT * span], f32, name=f"W{g}")
                  for g in range(n_wbufs)]

            # Loads mostly up front; a group's two halves ride DIFFERENT
            # rings so each group completes one ring-drain period after the
            # last. The LAST group's loads are emitted after group 0's
            # stores, so the first stores interleave with the load tail
            # (mixed reads+writes beat the write-only ring rate).
            def issue_load(g):
                for j in range(GT):
                    ring = nc.sync if j == 0 else nc.scalar
                    r0 = (g * GT + j) * P
                    rows = min(P, n - r0)
                    ring.dma_start(Rs[g][:rows, j * n:j * n + n],
                                   rho[r0:r0 + rows, :])

            for g in range(max(1, n_groups - 1)):
                issue_load(g)

            # Gap zeroing: W0/W1 memsets all complete before Vector's
            # first copy burst (GpSimd never memsets concurrently with
            # Vector copies — shared-SBUF-port stalls). W2 is zeroed by
            # Vector itself in its natural idle window after group 1's
            # copies, so group 2 does not wait on group 0's store drain.
            nc.vector.memset(Ws[0][:, 0:span], 0.0)
            nc.gpsimd.memset(Ws[0][:, span:GT * span], 0.0)
            if n_wbufs > 1:
                nc.gpsimd.memset(Ws[1][:, 0:span], 0.0)
                nc.gpsimd.memset(Ws[1][:, span:GT * span], 0.0)

            n_store = 0
            for g in range(n_groups):
                W, R = Ws[g % n_wbufs], Rs[g]

                if g == 2 and n_wbufs > 2:
                    nc.vector.memset(Ws[2][:, 0:span], 0.0)
                    nc.vector.memset(Ws[2][:, span:GT * span], 0.0)

                if g == 0:
                    # Group 0 is the store-stream critical path: copy and
                    # store PER TILE (3-dim APs) so tile 0's stores unlock
                    # as soon as its own half-load lands, ~1-2 us before
                    # tile 1's half arrives on the other ring.
                    for j in range(GT):
                        for d0, s0, ds, ss, cnt, L in pairs:
                            dst = bass.AP(W.tensor,
                                          W.offset + j * span + (d0 - c0),
                                          [[W.ap[0][0], P], [ds, cnt],
                                           [1, L]])
                            src = bass.AP(R.tensor, R.offset + j * n + s0,
                                          [[R.ap[0][0], P], [ss, cnt],
                                           [1, L]])
                            nc.vector.tensor_copy(dst, src)
                        r0 = (g * GT + j) * P
                        rows = min(P, n - r0)
                        for dr, sr, L in _runs(idx[r0:r0 + rows],
                                               range(rows)):
                            ring = (nc.sync if n_store % 2 == 0
                                    else nc.scalar)
                            n_store += 1
                            ring.dma_start(
                                out[dr:dr + L, c0:c1],
                                W[sr:sr + L, j * span:j * span + span])
                    if n_groups > 1:
                        issue_load(n_groups - 1)
                    continue

                # One 4-dim Vector copy per run-pair covers both tiles of
                # the group ([128 parts][GT tiles][2 runs][width]).
                for d0, s0, ds, ss, cnt, L in pairs:
                    dst = bass.AP(W.tensor, W.offset + (d0 - c0),
                                  [[W.ap[0][0], P], [span, GT],
                                   [ds, cnt], [1, L]])
                    src = bass.AP(R.tensor, R.offset + s0,
                                  [[R.ap[0][0], P], [n, GT],
                                   [ss, cnt], [1, L]])
                    nc.vector.tensor_copy(dst, src)

                # Row runs: consecutive rho rows with consecutive target rows
                # share one store DMA, alternating between the two rings.
                # (Routing stores through SWDGE as a third queue measured
                # ~10 us WORSE: Q7 descriptor generation serializes; merging
                # row-run pairs into 3-dim-AP stores lowered incorrectly.)
                for j in range(GT):
                    r0 = (g * GT + j) * P
                    rows = min(P, n - r0)
                    for dr, sr, L in _runs(idx[r0:r0 + rows], range(rows)):
                        ring = nc.sync if n_store % 2 == 0 else nc.scalar
                        n_store += 1
                        ring.dma_start(out[dr:dr + L, c0:c1],
                                       W[sr:sr + L, j * span:j * span + span])

    nc.compile()
    return nc


def kernel(input_state, fock_idx, fock_dim):
    input_state = np.asarray(input_state)
    idx = np.asarray(fock_idx).astype(np.int64)
    D = int(fock_dim)
    B, n, _ = input_state.shape

    nc = _build(idx, D, n)

    out = np.empty((B, D, D), dtype=input_state.dtype)
    for start in range(0, B, N_CORES):
        stop = min(start + N_CORES, B)
        in_maps = [
            {"rho": np.ascontiguousarray(input_state[b], dtype=np.float32)}
            for b in range(start, stop)
        ]
        res = run_bass_kernel_spmd(nc, in_maps,
                                   core_ids=list(range(stop - start)))
        for k, b in enumerate(range(start, stop)):
            out[b] = res.results[k]["out"]
    return out
